# revision 1
# baseline (speedup 1.0000x reference)
"""GQA causal attention (B=2, S=2048, H=2048, 32 Q heads / 8 KV heads, hd=64)
as an 8-way tensor-parallel Trainium2 Bass kernel.

Sharding: heads. Each NeuronCore gets 4 Q heads + their KV head (Wq/Wk/Wv
column slices, Wo row slice), computes a partial output over the full batch,
and the host sums the 8 partials (the Wo all-reduce done host-side).

Per-core dataflow (everything d-major / transposed so no on-device transposes
of activations are needed; host passes hidden pre-transposed):
    Q_T  = (Wq_c * scale)^T @ hidden^T        [256, B*S]
    KK_T = [Wk_c|Wk_c]^T @ hidden^T           [128, B*S] (duplicated halves so
                                              odd heads run on PE rows 64-127)
    V_T  = Wv_c^T @ hidden^T --PE-transpose-> V_aug [B*S, 65] (ones column
                                              accumulates the softmax denom)
    S_T[k,q] = K_T(chunk)^T x Q_T             only causal (lower) k-chunks
    P_T  = exp(S_T + tri-mask on diagonal chunks)      (no max-subtraction:
                                              scores are O(+-10), exp is safe)
    ctx_aug = V_aug^T @ P_T                   [65, q]; row 64 = denominator
    ctx  = ctx_aug[:64] * recip(denom)        stacked [256, q]
    out_partial = ctx^T @ Wo_c                [B*S, 2048]

All matmuls run as float32r (full-rate 1-cycle/row PE mode for fp32 data,
~1.5e-4 relative error measured on HW).
"""

import sys

for _p in ("/root/.axon_site", "/root/.axon_site/_ro/trn_rl_repo",
           "/root/.axon_site/_ro/pypackages", "/opt/trn_rl_repo", "/opt/pypackages"):
    if _p not in sys.path:
        sys.path.append(_p)

from contextlib import ExitStack

import numpy as np

import concourse.bass as bass  # noqa: F401
import concourse.tile as tile
from concourse import bacc, mybir
from concourse.bass_utils import run_bass_kernel_spmd

F32 = mybir.dt.float32
F32R = mybir.dt.float32r
P = 128
KC = 128
N_CORES = 8
HD = 64
NEG = -1e9

TRACE = False            # test harness flips this for NTFF profiling
TRACE_CORES = None
LAST_RESULT = None       # BassKernelResults of the last run (for the harness)

_nc_cache = {}


def build_attn_core(B=2, S=2048, H=2048, NHL=4, mask_mode="causal", QT=512,
                    debug_dump=False):
    """Build + bass-compile the per-core program.

    DRAM inputs (per core):
      ht  [H, B*S] f32r   hidden transposed      wq [H, NHL*HD] f32r (pre-scaled)
      wkv [H, 2*HD] f32r  [Wk_c | Wv_c]          wo [NHL*HD, H] f32r
      tri [KC, KC] f32    transposed causal block mask (tri[k,q]=0 iff k<=q)
      maskt [B, S, S] f32 (only mask_mode=="full") additive mask transposed
    Output: out_p [B*S, H] f32.
    """
    NQ = B * S
    CL = NHL * HD
    assert H % P == 0 and S % QT == 0 and QT % KC == 0 and NQ % QT == 0
    NHC = H // P
    NCC = CL // P
    QPB = S // QT
    KPB = S // KC
    DPT = QT // KC
    assert NHL % 2 == 0

    nc = bacc.Bacc("TRN2", target_bir_lowering=False, debug=False)

    ht = nc.dram_tensor("ht", [H, NQ], F32R, kind="ExternalInput").ap()
    wq = nc.dram_tensor("wq", [H, CL], F32R, kind="ExternalInput").ap()
    wkv = nc.dram_tensor("wkv", [H, 2 * HD], F32R, kind="ExternalInput").ap()
    wo = nc.dram_tensor("wo", [CL, H], F32R, kind="ExternalInput").ap()
    tri = nc.dram_tensor("tri", [KC, KC], F32, kind="ExternalInput").ap()
    ones = nc.dram_tensor("ones", [P, NQ // KC], F32R, kind="ExternalInput").ap()
    if mask_mode == "full":
        maskt = nc.dram_tensor("maskt", [B, S, S], F32, kind="ExternalInput").ap()
    out_p = nc.dram_tensor("out_p", [NQ, H], F32, kind="ExternalOutput").ap()

    with tile.TileContext(nc) as tc, ExitStack() as ctx:
        # ---- persistent SBUF ----
        pers = ctx.enter_context(tc.tile_pool(name="pers", bufs=1))
        wq_sb = pers.tile([P, NHC, CL], F32R, tag="wq")
        nc.sync.dma_start(wq_sb[:], wq.rearrange("(o p) m -> p o m", p=P))
        wkv_sb = pers.tile([P, NHC, 2 * HD], F32R, tag="wkv")
        nc.sync.dma_start(wkv_sb[:], wkv.rearrange("(o p) m -> p o m", p=P))
        wo_sb = pers.tile([P, NCC, H], F32R, tag="wo")
        nc.sync.dma_start(wo_sb[:], wo.rearrange("(o p) m -> p o m", p=P))
        tri_sb = pers.tile([KC, KC], F32, tag="tri")
        nc.sync.dma_start(tri_sb[:], tri)

        # identity (fp32) for PE transposes: keep diagonal 1.0, fill 0 off it
        ident = pers.tile([P, P], F32, tag="ident")
        nc.gpsimd.memset(ident[:], 1.0)
        nc.gpsimd.affine_select(
            out=ident[:], in_=ident[:],
            compare_op=mybir.AluOpType.is_equal, fill=0.0,
            base=0, pattern=[[-1, P]], channel_multiplier=1,
        )

        qt_sb = [pers.tile([P, NQ], F32R, tag=f"qt{c}", name=f"qt{c}")
                 for c in range(NCC)]
        kt_sb = pers.tile([P, NQ], F32R, tag="kt")          # [K_T ; K_T]
        v_sb = pers.tile([P, NQ // KC, HD + 1], F32R, tag="v")
        ctx_sb = pers.tile([P, NCC, QT], F32R, tag="ctx")

        # denom ones column (DMA'd: gpsimd memset can't write f32r)
        nc.sync.dma_start(v_sb[:, :, HD], ones)

        # ---- pools ----
        hpool = ctx.enter_context(tc.tile_pool(name="hpool", bufs=4))
        vtmp_pool = ctx.enter_context(tc.tile_pool(name="vtmp", bufs=2))
        pt_pool = ctx.enter_context(tc.tile_pool(name="pt", bufs=4))
        npool = ctx.enter_context(tc.tile_pool(name="npool", bufs=4))
        opool = ctx.enter_context(tc.tile_pool(name="opool", bufs=3))
        if mask_mode == "full":
            mpool = ctx.enter_context(tc.tile_pool(name="mpool", bufs=4))

        psA = ctx.enter_context(tc.tile_pool(name="psA", bufs=2, space="PSUM"))
        psB = ctx.enter_context(tc.tile_pool(name="psB", bufs=1, space="PSUM"))
        psS = ctx.enter_context(tc.tile_pool(name="psS", bufs=2, space="PSUM"))
        psC = ctx.enter_context(tc.tile_pool(name="psC", bufs=1, space="PSUM"))
        psO = ctx.enter_context(tc.tile_pool(name="psO", bufs=2, space="PSUM"))

        if debug_dump:
            dbg_qt = nc.dram_tensor("dbg_qt", [NCC, P, NQ], F32, kind="ExternalOutput").ap()
            dbg_kt = nc.dram_tensor("dbg_kt", [P, NQ], F32, kind="ExternalOutput").ap()
            dbg_v = nc.dram_tensor("dbg_v", [P, NQ // KC, HD + 1], F32, kind="ExternalOutput").ap()

        # ================= Phase A: projections =================
        NQT = NQ // QT
        for qt in range(NQT):
            q0 = qt * QT
            pq = [psA.tile([P, QT], F32, tag="pq", name=f"pq{i}") for i in range(NCC)]
            pkv = psB.tile([P, QT], F32, tag="pkv")
            for hc in range(NHC):
                h_t = hpool.tile([P, QT], F32R, tag="h")
                nc.sync.dma_start(h_t[:], ht[hc * P:(hc + 1) * P, q0:q0 + QT])
                fl = dict(start=(hc == 0), stop=(hc == NHC - 1))
                for cc in range(NCC):
                    nc.tensor.matmul(pq[cc][:], wq_sb[:, hc, cc * P:(cc + 1) * P],
                                     h_t[:], **fl)
                nc.tensor.matmul(pkv[:], wkv_sb[:, hc, :], h_t[:], **fl)
            for cc in range(NCC):
                nc.vector.tensor_copy(qt_sb[cc][:, q0:q0 + QT], pq[cc][:])
            # K_T rows 0-63; duplicate to 64-127 via SBUF->SBUF DMA
            nc.vector.tensor_copy(kt_sb[:HD, q0:q0 + QT], pkv[:HD, :])
            nc.sync.dma_start(kt_sb[HD:2 * HD, q0:q0 + QT], kt_sb[:HD, q0:q0 + QT])
            # V_T chunk -> PE-transpose into v_sb (natural [k, d] layout)
            vtmp = vtmp_pool.tile([P, QT], F32, tag="vt")
            nc.vector.tensor_copy(vtmp[HD:2 * HD, :], pkv[HD:2 * HD, :])
            for s4 in range(DPT):
                tp = psS.tile([P, QT], F32, tag="ps_s", name="tp")
                nc.tensor.transpose(
                    tp[:, :HD],
                    vtmp[HD:2 * HD, s4 * KC:(s4 + 1) * KC],
                    ident[HD:2 * HD, HD:2 * HD],
                )
                nc.vector.tensor_copy(v_sb[:, qt * DPT + s4, :HD], tp[:, :HD])

        if debug_dump:
            for c in range(NCC):
                nc.sync.dma_start(dbg_qt[c], qt_sb[c][:].bitcast(F32))
            nc.sync.dma_start(dbg_kt[:], kt_sb[:].bitcast(F32))
            nc.sync.dma_start(dbg_v[:], v_sb[:].bitcast(F32))

        # ================= Phase B: attention + out proj =================
        for b in range(B):
            for qtb in range(QPB):
                q0b = qtb * QT
                q0 = b * S + q0b
                nkc = (qtb + 1) * DPT if mask_mode == "causal" else KPB
                for h in range(NHL):
                    hb = (h % 2) * HD
                    cc = h // 2
                    cps = psC.tile([HD + 1, QT], F32, tag="ctx_ps")
                    for kc in range(nkc):
                        kcg = b * KPB + kc
                        diag_off = kc * KC - q0b
                        sps = psS.tile([P, QT], F32, tag="ps_s", name="sps")
                        nc.tensor.matmul(
                            sps[:],
                            kt_sb[hb:hb + HD, kcg * KC:(kcg + 1) * KC],
                            qt_sb[cc][hb:hb + HD, q0:q0 + QT],
                            start=True, stop=True,
                        )
                        pt = pt_pool.tile([P, QT], F32R, tag="pt")
                        if mask_mode == "full":
                            mt = mpool.tile([KC, QT], F32, tag="mt")
                            nc.sync.dma_start(
                                mt[:], maskt[b, kc * KC:(kc + 1) * KC, q0b:q0b + QT])
                            nc.vector.tensor_add(sps[:], sps[:], mt[:])
                            nc.scalar.activation(
                                pt[:], sps[:], mybir.ActivationFunctionType.Exp)
                        elif mask_mode == "causal" and diag_off >= 0:
                            # diagonal chunk: cols < diag_off fully masked,
                            # [diag_off, diag_off+KC) triangular, rest free
                            nc.vector.tensor_add(
                                sps[:, diag_off:diag_off + KC],
                                sps[:, diag_off:diag_off + KC],
                                tri_sb[:],
                            )
                            if diag_off > 0:
                                # cols left of the diagonal are fully masked:
                                # P = scores*0 = 0 (DVE can write f32r; memset can't)
                                nc.vector.tensor_scalar_mul(
                                    pt[:, :diag_off], sps[:, :diag_off], 0.0)
                            nc.scalar.activation(
                                pt[:, diag_off:], sps[:, diag_off:],
                                mybir.ActivationFunctionType.Exp,
                            )
                        else:
                            nc.scalar.activation(
                                pt[:], sps[:], mybir.ActivationFunctionType.Exp)
                        nc.tensor.matmul(
                            cps[:], v_sb[:, kcg, :], pt[:],
                            start=(kc == 0), stop=(kc == nkc - 1),
                        )
                    # normalize: ctx[:64] * recip(denom row). Denom is on PSUM
                    # partition 64; DVE is partition-locked, so recip stays on
                    # partition 64, a 2KB DMA moves it to partition 0, and
                    # gpsimd broadcasts it across partitions 0-63.
                    recip = npool.tile([P, QT], F32, tag="recip")
                    nc.vector.reciprocal(recip[HD:HD + 1, :], cps[HD:HD + 1, :])
                    nc.sync.dma_start(recip[0:1, :], recip[HD:HD + 1, :])
                    bcast = npool.tile([HD, QT], F32, tag="bcast")
                    nc.gpsimd.partition_broadcast(bcast[:], recip[0:1, :])
                    if h % 2 == 0:
                        nc.vector.tensor_mul(ctx_sb[:HD, cc, :], cps[:HD, :], bcast[:])
                    else:
                        ctmp = npool.tile([HD, QT], F32R, tag="ctmp")
                        nc.vector.tensor_mul(ctmp[:], cps[:HD, :], bcast[:])
                        nc.sync.dma_start(ctx_sb[HD:2 * HD, cc, :], ctmp[:])
                # ---- Wo ----
                ET = min(512, H)
                for qc in range(QT // P):
                    for et in range(H // ET):
                        po = psO.tile([P, ET], F32, tag="po")
                        for cc in range(NCC):
                            nc.tensor.matmul(
                                po[:],
                                ctx_sb[:, cc, qc * P:(qc + 1) * P],
                                wo_sb[:, cc, et * ET:(et + 1) * ET],
                                start=(cc == 0), stop=(cc == NCC - 1),
                            )
                        ob = opool.tile([P, ET], F32, tag="ob")
                        nc.vector.tensor_copy(ob[:], po[:])
                        nc.sync.dma_start(
                            out_p[q0 + qc * P:q0 + (qc + 1) * P, et * ET:(et + 1) * ET],
                            ob[:],
                        )

    nc.compile()
    return nc


def _detect_mask_mode(m, S):
    if not np.any(m):
        return "zeros"
    b0 = np.asarray(m[0, 0])
    qi = np.arange(S)
    tl = qi[None, :] <= qi[:, None]
    if (b0[tl] == 0.0).all() and (b0[~tl] <= -1e8).all() and (m == b0).all():
        return "causal"
    return "full"


def shard_inputs(hidden_states, attention_mask, Wq, Wk, Wv, Wo, mask_mode):
    B, S, H = hidden_states.shape
    NH = Wq.shape[1] // HD
    NKV = Wk.shape[1] // HD
    NHL = NH // N_CORES
    scale = np.float32(1.0 / np.sqrt(HD))

    ht = np.ascontiguousarray(
        hidden_states.reshape(B * S, H).T.astype(np.float32))
    if mask_mode == "causal":
        tri = np.ascontiguousarray(attention_mask[0, 0, :KC, :KC].T.astype(np.float32))
    else:
        tri = np.zeros((KC, KC), np.float32)
    if mask_mode == "full":
        maskt = np.ascontiguousarray(
            np.asarray(attention_mask)[:, 0].transpose(0, 2, 1).astype(np.float32))

    ones_np = np.ones((P, (B * S) // KC), np.float32)
    in_maps = []
    for c in range(N_CORES):
        wq_c = np.ascontiguousarray(
            Wq[:, c * NHL * HD:(c + 1) * NHL * HD].astype(np.float32) * scale)
        kv0 = c * (NKV // N_CORES) * HD
        wkv_c = np.ascontiguousarray(np.concatenate(
            [Wk[:, kv0:kv0 + HD], Wv[:, kv0:kv0 + HD]], axis=1).astype(np.float32))
        wo_c = np.ascontiguousarray(
            Wo[c * NHL * HD:(c + 1) * NHL * HD, :].astype(np.float32))
        im = {"ht": ht, "wq": wq_c, "wkv": wkv_c, "wo": wo_c, "tri": tri,
              "ones": ones_np}
        if mask_mode == "full":
            im["maskt"] = maskt
        in_maps.append(im)
    return in_maps, NHL


def kernel(hidden_states, attention_mask, Wq, Wk, Wv, Wo):
    global LAST_RESULT
    hidden_states = np.asarray(hidden_states, dtype=np.float32)
    attention_mask = np.asarray(attention_mask, dtype=np.float32)
    Wq, Wk, Wv, Wo = (np.asarray(w, dtype=np.float32) for w in (Wq, Wk, Wv, Wo))
    B, S, H = hidden_states.shape

    mask_mode = _detect_mask_mode(attention_mask, S)
    in_maps, NHL = shard_inputs(hidden_states, attention_mask, Wq, Wk, Wv, Wo,
                                mask_mode)

    key = (B, S, H, NHL, mask_mode)
    if key not in _nc_cache:
        _nc_cache[key] = build_attn_core(B=B, S=S, H=H, NHL=NHL,
                                         mask_mode=mask_mode)
    nc = _nc_cache[key]

    res = run_bass_kernel_spmd(nc, in_maps, core_ids=list(range(N_CORES)),
                               trace=TRACE, trace_cores=TRACE_CORES)
    LAST_RESULT = res

    out = res.results[0]["out_p"].astype(np.float32).copy()
    for c in range(1, N_CORES):
        out += res.results[c]["out_p"]
    return out.reshape(B, S, H)



# revision 17
# speedup vs baseline: 1.7635x; 1.7635x over previous
"""GQA causal attention (B=2, S=2048, H=2048, 32 Q heads / 8 KV heads, hd=64)
as an 8-way batch x head tensor-parallel Trainium2 Bass kernel.

Sharding: core c = (batch b = c//4, group g = c%4). Each core gets one batch
element, 8 Q heads (two GQA groups) and their 2 KV heads; Wq/Wk/Wv column
slices, Wo row slice. Host sums the 4 partials per batch (the Wo all-reduce).

Head layout trick: per-core Q-head columns are permuted host-side so chunk cc
of qt_sb holds head cc on partitions 0-63 (KV head 0) and head 4+cc on
partitions 64-127 (KV head 1). K^T for KV head j sits on partitions 64j..64j+63
of kt_sb, so every scores matmul has matching base partitions with no K
duplication. Wo rows carry the same permutation.

Per-core dataflow (d-major; host passes hidden pre-transposed):
    Q_T  = (Wq_c * scale)^T @ hidden^T            [512, S]
    K_T  = [Wk0|Wk1]^T @ hidden^T                 [128, S]
    V_T  -> PE-transpose -> V_aug [S-chunked, 65] bf16 (ones col = denom)
    S_T[k,q] = tri_mask (identity-stationary matmul) + K_T(chunk)^T x Q_T
               (diagonal chunks first, extents trimmed to the causal region)
    P_T  = exp(S_T)  bf16                          (scores are O(+-10))
    ctx_aug = V_aug^T @ P_T                        [65, q]; row 64 = denom
    ctx  = ctx_aug[:64] * recip_approx(denom)      broadcast via gpsimd
    out_partial = ctx^T @ Wo_c                     [S, 2048], accumulated by qtb
Wo for q-block i is emitted after attention of q-block i+1 so the scheduler
keeps the PE stream dense (TRN2 PE clock ramps only while continuously busy).
"""

import sys

for _p in ("/root/.axon_site", "/root/.axon_site/_ro/trn_rl_repo",
           "/root/.axon_site/_ro/pypackages", "/opt/trn_rl_repo", "/opt/pypackages"):
    if _p not in sys.path:
        sys.path.append(_p)

from contextlib import ExitStack

import numpy as np

import concourse.bass as bass  # noqa: F401
import concourse.tile as tile
from concourse import bacc, mybir
from concourse.bass_utils import run_bass_kernel_spmd

F32 = mybir.dt.float32
F32R = mybir.dt.float32r
BF16 = mybir.dt.bfloat16
P = 128
KC = 128
QT = 512
N_CORES = 8
HD = 64
NEG = -1e9

TRACE = False            # test harness flips this for NTFF profiling
TRACE_CORES = None
LAST_RESULT = None       # BassKernelResults of the last run (for the harness)

_nc_cache = {}


def build_attn_core(S=2048, H=2048, NH=8, mask_mode="causal", debug_dump=False):
    """Build + bass-compile the per-core program (one batch element).

    DRAM inputs (per core):
      ht  [H, S] f32r    hidden transposed       wq [H, NH*HD] f32r (pre-scaled,
                                                 head-permuted: chunk cc = heads
                                                 (cc, 4+cc))
      wkv [H, 4*HD] f32r [K0|K1|V0|V1]           wo [NH*HD, H] f32r (row-permuted)
      trif [KC, QT] f32r [tri | zeros]: trif[k,j] = -1e9 if k > j else 0
      identr [P, P] f32r identity                zrow [1, HD+1] f32r zeros
      maskt [S, S] f32   (mask_mode=="full" only) additive mask transposed
    Output: out_p [S, H] f32.
    """
    NKV = 2
    CL = NH * HD                       # 512 q cols
    NCC = CL // P                      # 4 qt chunks
    NHC = H // P                       # 16 contraction chunks
    NT = S // QT                       # 4 token tiles / q blocks
    KPB = S // KC                      # 16 k chunks
    DPT = QT // KC                     # 4 k chunks per q block
    assert NH == 8 and S % QT == 0 and H % P == 0

    nc = bacc.Bacc("TRN2", target_bir_lowering=False, debug=False)

    ht = nc.dram_tensor("ht", [H, S], F32R, kind="ExternalInput").ap()
    wq = nc.dram_tensor("wq", [H, CL], F32R, kind="ExternalInput").ap()
    wkv = nc.dram_tensor("wkv", [H, 4 * HD], F32R, kind="ExternalInput").ap()
    wo = nc.dram_tensor("wo", [CL, H], F32R, kind="ExternalInput").ap()
    trif = nc.dram_tensor("trif", [KC, QT], F32R, kind="ExternalInput").ap()
    identr = nc.dram_tensor("identr", [P, P], F32R, kind="ExternalInput").ap()
    zrow = nc.dram_tensor("zrow", [1, HD + 1], F32R, kind="ExternalInput").ap()
    if mask_mode == "full":
        maskt = nc.dram_tensor("maskt", [S, S], F32, kind="ExternalInput").ap()
    out_p = nc.dram_tensor("out_p", [S, H], F32, kind="ExternalOutput").ap()
    if debug_dump:
        dbg_qt = nc.dram_tensor("dbg_qt", [NCC, P, S], F32, kind="ExternalOutput").ap()
        dbg_kt = nc.dram_tensor("dbg_kt", [P, S], F32, kind="ExternalOutput").ap()
        dbg_v = nc.dram_tensor("dbg_v", [P, S // KC, NKV, HD + 1], F32,
                               kind="ExternalOutput").ap()
        dbg_ctx = nc.dram_tensor("dbg_ctx", [2, P, NCC, QT], F32,
                                 kind="ExternalOutput").ap()

    with tile.TileContext(nc) as tc, ExitStack() as ctx:
        # ---- persistent SBUF ----
        pers = ctx.enter_context(tc.tile_pool(name="pers", bufs=1))
        wq_sb = pers.tile([P, NHC, CL], F32R, tag="wq")
        wkv_sb = pers.tile([P, NHC, 4 * HD], F32R, tag="wkv")
        wq_r = wq.rearrange("(o p) m -> p o m", p=P)
        wkv_r = wkv.rearrange("(o p) m -> p o m", p=P)
        for hc in range(NHC):
            nc.sync.dma_start(wq_sb[:, hc], wq_r[:, hc])
            nc.sync.dma_start(wkv_sb[:, hc], wkv_r[:, hc])
        trif_sb = pers.tile([KC, QT], F32R, tag="trif")
        nc.sync.dma_start(trif_sb[:], trif)
        idr_sb = pers.tile([P, P], F32R, tag="identr")
        nc.sync.dma_start(idr_sb[:], identr)
        zrow_sb = pers.tile([1, HD + 1], F32R, tag="zrow")
        nc.sync.dma_start(zrow_sb[:], zrow)
        wo_sb = pers.tile([P, NCC, H], F32R, tag="wo")
        nc.sync.dma_start(wo_sb[:], wo.rearrange("(o p) m -> p o m", p=P))

        # fp32 identity for PE transposes (gpsimd memset can write f32)
        ident = pers.tile([P, P], F32, tag="ident")
        nc.gpsimd.memset(ident[:], 1.0)
        nc.gpsimd.affine_select(
            out=ident[:], in_=ident[:],
            compare_op=mybir.AluOpType.is_equal, fill=0.0,
            base=0, pattern=[[-1, P]], channel_multiplier=1,
        )

        qt_sb = pers.tile([P, NCC, S], F32R, tag="qt")
        kt_sb = pers.tile([P, S], F32R, tag="kt")
        v_sb = pers.tile([P, KPB, NKV, HD + 1], BF16, tag="v")
        # ones column for the softmax denominator: fill everything with 1.0,
        # proj drains then overwrite cols 0..63
        nc.gpsimd.memset(v_sb[:], 1.0)

        # ---- pools ----
        hpool = ctx.enter_context(tc.tile_pool(name="hpool", bufs=4))
        vtmp_pool = ctx.enter_context(tc.tile_pool(name="vtmp", bufs=2))
        pt_pool = ctx.enter_context(tc.tile_pool(name="pt", bufs=4))
        npool = ctx.enter_context(tc.tile_pool(name="npool", bufs=3))
        cxpool = ctx.enter_context(tc.tile_pool(name="cxpool", bufs=2))
        if mask_mode == "full":
            mpool = ctx.enter_context(tc.tile_pool(name="mpool", bufs=4))

        # PSUM: exactly 8 banks across both phases
        psS = ctx.enter_context(tc.tile_pool(name="psS", bufs=2, space="PSUM"))
        psC = ctx.enter_context(tc.tile_pool(name="psC", bufs=2, space="PSUM"))
        psO = ctx.enter_context(tc.tile_pool(name="psO", bufs=2, space="PSUM"))

        ctx_sb = [cxpool.tile([P, NCC, QT], F32R, tag="ctx", name=f"ctx{i}")
                  for i in range(2)]

        # ================= projections (one q tile) =================
        # psS slots ([128,1024] = 2 banks) hold Q chunk pairs; psO slots
        # ([128,512]) hold the K and V chunks; psC slots host the transposes.
        # Tiles >= 1 are emitted interleaved with attention of tile-1 (their
        # drains go on DVE; Act is the attention-window co-bottleneck).
        def proj_tile(t):
            q0 = t * QT
            pq01 = psS.tile([P, 2 * QT], F32, tag="sps", name="pq01")
            pq23 = psS.tile([P, 2 * QT], F32, tag="sps", name="pq23")
            pk = psO.tile([P, QT], F32, tag="po", name="pk")
            pv = psO.tile([P, QT], F32, tag="po", name="pv")
            for hc in range(NHC):
                h_t = hpool.tile([P, QT], F32R, tag="h")
                nc.sync.dma_start(h_t[:], ht[hc * P:(hc + 1) * P, q0:q0 + QT])
                fl = dict(start=(hc == 0), stop=(hc == NHC - 1))
                nc.tensor.matmul(pq01[:, :QT], wq_sb[:, hc, 0:P], h_t[:], **fl)
                nc.tensor.matmul(pq01[:, QT:], wq_sb[:, hc, P:2 * P], h_t[:], **fl)
                nc.tensor.matmul(pq23[:, :QT], wq_sb[:, hc, 2 * P:3 * P], h_t[:], **fl)
                nc.tensor.matmul(pq23[:, QT:], wq_sb[:, hc, 3 * P:4 * P], h_t[:], **fl)
                nc.tensor.matmul(pk[:], wkv_sb[:, hc, 0:P], h_t[:], **fl)
                nc.tensor.matmul(pv[:], wkv_sb[:, hc, P:2 * P], h_t[:], **fl)
            cp = nc.scalar.copy if t == 0 else nc.vector.tensor_copy
            cp(qt_sb[:, 0, q0:q0 + QT], pq01[:, :QT])
            cp(qt_sb[:, 1, q0:q0 + QT], pq01[:, QT:])
            cp(qt_sb[:, 2, q0:q0 + QT], pq23[:, :QT])
            cp(qt_sb[:, 3, q0:q0 + QT], pq23[:, QT:])
            cp(kt_sb[:, q0:q0 + QT], pk[:])
            vtmp = vtmp_pool.tile([P, QT], F32, tag="vt")
            nc.vector.tensor_copy(vtmp[:], pv[:])
            # V_T chunk -> PE-transpose into v_sb (natural [k, d] layout)
            for j in range(NKV):
                for s4 in range(DPT):
                    tp = psC.tile([P, HD], F32, tag="cps", name="tp")
                    nc.tensor.transpose(
                        tp[:, :HD],
                        vtmp[HD * j:HD * (j + 1), s4 * KC:(s4 + 1) * KC],
                        ident[HD * j:HD * (j + 1), HD * j:HD * (j + 1)],
                    )
                    nc.vector.tensor_copy(v_sb[:, t * DPT + s4, j, :HD], tp[:, :HD])

        def debug_dumps():
            for c in range(NCC):
                nc.sync.dma_start(dbg_qt[c], qt_sb[:, c, :].bitcast(F32))
            nc.sync.dma_start(dbg_kt[:], kt_sb[:].bitcast(F32))
            dpool = ctx.enter_context(tc.tile_pool(name="dpool", bufs=1))
            dvt = dpool.tile([P, KPB, NKV, HD + 1], F32, tag="dvt")
            nc.vector.tensor_copy(dvt[:], v_sb[:])
            nc.sync.dma_start(dbg_v[:], dvt[:])

        # ================= Phase B: attention, Wo lagged one q-block =======
        def emit_wo(qtb):
            src = ctx_sb[qtb % 2]
            q0 = qtb * QT
            for tc_ in range(QT // P):
                for et in range(H // QT):
                    po = psO.tile([P, QT], F32, tag="po", name="po")
                    for cc in range(NCC):
                        nc.tensor.matmul(
                            po[:],
                            src[:, cc, tc_ * P:(tc_ + 1) * P],
                            wo_sb[:, cc, et * QT:(et + 1) * QT],
                            start=(cc == 0), stop=(cc == NCC - 1),
                        )
                    ob = npool.tile([P, QT], F32, tag="ob")
                    nc.vector.tensor_copy(ob[:], po[:])
                    nc.sync.dma_start(
                        out_p[q0 + tc_ * P:q0 + (tc_ + 1) * P,
                              et * QT:(et + 1) * QT],
                        ob[:],
                    )

        def attn_head(qtb, h):
            q0 = qtb * QT
            r = h // 4               # kv slot == partition half
            cc = h % 4               # qt chunk
            rows = slice(HD * r, HD * (r + 1))
            cps = psC.tile([HD + 1, QT], F32, tag="cps", name="cps")

            if mask_mode == "causal":
                # (kc, local q offset, extent, in-tile offset); diagonal
                # chunks first, paired (d0,d3) and (d1,d2) so each pair's
                # exp covers ONE contiguous range
                d = qtb * DPT
                pairs = [
                    [(d + 0, 0, QT, 0), (d + 3, 3 * KC, KC, QT)],
                    [(d + 1, KC, QT - KC, KC), (d + 2, 2 * KC, 2 * KC, QT)],
                ]
                ints = [(kc, 0, QT, (i % 2) * QT)
                        for i, kc in enumerate(range(qtb * DPT))]
            else:
                ints = [(kc, 0, QT, (i % 2) * QT)
                        for i, kc in enumerate(range(KPB))]
                pairs = []
            pairs += [ints[i:i + 2] for i in range(0, len(ints), 2)]
            nch = sum(len(p) for p in pairs)

            ci = 0
            for pair in pairs:
                sps = psS.tile([P, 2 * QT], F32, tag="sps", name="sps")
                pt = pt_pool.tile([P, 2 * QT], BF16, tag="pt")
                for kc, qoff, ext, off in pair:
                    diag = mask_mode == "causal" and kc >= qtb * DPT
                    if diag:
                        # tri mask first (start=True zeroes the region),
                        # then accumulate the scores on top
                        nc.tensor.matmul(
                            sps[:, off:off + ext], idr_sb[:],
                            trif_sb[:, :ext], start=True, stop=False)
                    nc.tensor.matmul(
                        sps[:, off:off + ext],
                        kt_sb[rows, kc * KC:(kc + 1) * KC],
                        qt_sb[rows, cc, q0 + qoff:q0 + qoff + ext],
                        start=not diag, stop=True,
                    )
                    if mask_mode == "full":
                        mt = mpool.tile([KC, QT], F32, tag="mt")
                        nc.sync.dma_start(
                            mt[:], maskt[kc * KC:(kc + 1) * KC, q0:q0 + QT])
                        nc.vector.tensor_add(sps[:, off:off + ext],
                                             sps[:, off:off + ext], mt[:])
                # one exp over the pair's contiguous range
                lo = min(off for _, _, _, off in pair)
                hi = max(off + ext for _, _, ext, off in pair)
                nc.scalar.activation(pt[:, lo:hi], sps[:, lo:hi],
                                     mybir.ActivationFunctionType.Exp)
                for kc, qoff, ext, off in pair:
                    nc.tensor.matmul(
                        cps[:, qoff:qoff + ext],
                        v_sb[:, kc, r, :],
                        pt[:, off:off + ext],
                        start=(ci == 0),
                        stop=(ci == nch - 1) and (qoff == 0),
                    )
                    ci += 1
            if mask_mode == "causal" and qtb == 0:
                # no full-width final chunk at qtb 0; close the accumulation
                # group over the whole bank with a zero matmul
                nc.tensor.matmul(
                    cps[:], zrow_sb[:], qt_sb[0:1, 0, q0:q0 + QT],
                    start=False, stop=True)

            # ---- normalize: ctx[:64] * recip(denom row 64) ----
            recip = npool.tile([P, QT + 2 * (QT // P)], F32, tag="recip")
            DW = QT // P
            # reshape the [1, QT] denom row to [P, QT/P] via DMA so the
            # partition-serial reciprocal runs 128-wide (~100ns vs ~4us)
            nc.vector.tensor_copy(recip[HD:HD + 1, :QT], cps[HD:HD + 1, :])
            nc.sync.dma_start(recip[:, QT:QT + DW], recip[HD:HD + 1, :QT])
            nc.vector.reciprocal(recip[:, QT + DW:], recip[:, QT:QT + DW])
            nc.sync.dma_start(recip[0:1, :QT], recip[:, QT + DW:])
            bcast = npool.tile([HD, QT], F32, tag="bcast")
            nc.gpsimd.partition_broadcast(bcast[:], recip[0:1, :QT])
            dst = ctx_sb[qtb % 2]
            if r == 0:
                nc.vector.tensor_mul(dst[:HD, cc, :], cps[:HD, :], bcast[:])
            else:
                ctmp = npool.tile([HD, QT], F32R, tag="ctmp")
                nc.vector.tensor_mul(ctmp[:], cps[:HD, :], bcast[:])
                nc.sync.dma_start(dst[HD:2 * HD, cc, :], ctmp[:])

        proj_tile(0)
        for qtb in range(NT):
            for h in range(NH):
                attn_head(qtb, h)
            if debug_dump and qtb < 2:
                nc.sync.dma_start(dbg_ctx[qtb], ctx_sb[qtb % 2][:].bitcast(F32))
            if qtb > 0:
                emit_wo(qtb - 1)
            if qtb + 1 < NT:
                proj_tile(qtb + 1)
        emit_wo(NT - 1)
        if debug_dump:
            debug_dumps()

    nc.compile()
    return nc


def _detect_mask_mode(m, S):
    if not np.any(m):
        return "zeros"
    b0 = np.asarray(m[0, 0])
    qi = np.arange(S)
    tl = qi[None, :] <= qi[:, None]
    if (b0[tl] == 0.0).all() and (b0[~tl] <= -1e8).all() and (m == b0).all():
        return "causal"
    return "full"


def shard_inputs(hidden_states, attention_mask, Wq, Wk, Wv, Wo, mask_mode):
    B, S, H = hidden_states.shape
    NH = Wq.shape[1] // HD             # 32 total
    NKV = Wk.shape[1] // HD            # 8 total
    G = 4                              # head-groups (cores per batch)
    NHL = NH // G                      # 8 q heads per core
    scale = np.float32(1.0 / np.sqrt(HD))

    # permutation: chunk cc holds q-heads (cc, 4+cc) -> [0,4,1,5,2,6,3,7]
    perm = [h for cc in range(NHL // 2) for h in (cc, cc + NHL // 2)]

    trif = np.zeros((KC, QT), np.float32)
    ki, qj = np.meshgrid(np.arange(KC), np.arange(KC), indexing="ij")
    trif[:, :KC] = np.where(ki > qj, NEG, 0.0).astype(np.float32)
    identr = np.eye(P, dtype=np.float32)
    zr = np.zeros((1, HD + 1), np.float32)

    in_maps = []
    for c in range(N_CORES):
        b, g = divmod(c, G)
        ht = np.ascontiguousarray(
            np.asarray(hidden_states[b]).T.astype(np.float32))
        heads = [g * NHL + perm[i] for i in range(NHL)]
        wq_c = np.ascontiguousarray(np.concatenate(
            [Wq[:, h * HD:(h + 1) * HD] for h in heads], axis=1)
            .astype(np.float32) * scale)
        kv0 = g * 2
        wkv_c = np.ascontiguousarray(np.concatenate(
            [Wk[:, kv0 * HD:(kv0 + 2) * HD],
             Wv[:, kv0 * HD:(kv0 + 2) * HD]], axis=1).astype(np.float32))
        wo_c = np.ascontiguousarray(np.concatenate(
            [Wo[h * HD:(h + 1) * HD, :] for h in heads], axis=0)
            .astype(np.float32))
        im = {"ht": ht, "wq": wq_c, "wkv": wkv_c, "wo": wo_c,
              "trif": trif, "identr": identr, "zrow": zr}
        if mask_mode == "full":
            im["maskt"] = np.ascontiguousarray(
                np.asarray(attention_mask)[b, 0].T.astype(np.float32))
        in_maps.append(im)
    return in_maps


def kernel(hidden_states, attention_mask, Wq, Wk, Wv, Wo):
    global LAST_RESULT
    hidden_states = np.asarray(hidden_states, dtype=np.float32)
    attention_mask = np.asarray(attention_mask, dtype=np.float32)
    Wq, Wk, Wv, Wo = (np.asarray(w, dtype=np.float32) for w in (Wq, Wk, Wv, Wo))
    B, S, H = hidden_states.shape

    mask_mode = _detect_mask_mode(attention_mask, S)
    in_maps = shard_inputs(hidden_states, attention_mask, Wq, Wk, Wv, Wo,
                           mask_mode)

    key = (B, S, H, mask_mode)
    if key not in _nc_cache:
        _nc_cache[key] = build_attn_core(S=S, H=H, NH=8, mask_mode=mask_mode)
    nc = _nc_cache[key]

    res = run_bass_kernel_spmd(nc, in_maps, core_ids=list(range(N_CORES)),
                               trace=TRACE, trace_cores=TRACE_CORES)
    LAST_RESULT = res

    out = np.zeros((B, S, H), np.float32)
    for c in range(N_CORES):
        out[c // 4] += res.results[c]["out_p"]
    return out


# revision 21
# speedup vs baseline: 1.7793x; 1.0090x over previous
"""GQA causal attention (B=2, S=2048, H=2048, 32 Q heads / 8 KV heads, hd=64)
as an 8-way batch x head tensor-parallel Trainium2 Bass kernel.

Sharding: core c = (batch b = c//4, group g = c%4). Each core gets one batch
element, 8 Q heads (two GQA groups) and their 2 KV heads; Wq/Wk/Wv column
slices, Wo row slice. Host sums the 4 partials per batch (the Wo all-reduce).

Head layout trick: per-core Q-head columns are permuted host-side so chunk cc
of qt_sb holds head cc on partitions 0-63 (KV head 0) and head 4+cc on
partitions 64-127 (KV head 1). K^T for KV head j sits on partitions 64j..64j+63
of kt_sb, so every scores matmul has matching base partitions with no K
duplication. Wo rows carry the same permutation.

Per-core dataflow (d-major; host passes hidden pre-transposed):
    Q_T  = (Wq_c * scale)^T @ hidden^T            [512, S]
    K_T  = [Wk0|Wk1]^T @ hidden^T                 [128, S]
    V_T  -> PE-transpose -> V_aug [S-chunked, 65] bf16 (ones col = denom)
    S_T[k,q] = tri_mask (identity-stationary matmul) + K_T(chunk)^T x Q_T
               (diagonal chunks first, extents trimmed to the causal region)
    P_T  = exp(S_T)  bf16                          (scores are O(+-10))
    ctx_aug = V_aug^T @ P_T                        [65, q]; row 64 = denom
    ctx  = ctx_aug[:64] * recip_approx(denom)      broadcast via gpsimd
    out_partial = ctx^T @ Wo_c                     [S, 2048], accumulated by qtb
Wo for q-block i is emitted after attention of q-block i+1 so the scheduler
keeps the PE stream dense (TRN2 PE clock ramps only while continuously busy).
"""

import sys

for _p in ("/root/.axon_site", "/root/.axon_site/_ro/trn_rl_repo",
           "/root/.axon_site/_ro/pypackages", "/opt/trn_rl_repo", "/opt/pypackages"):
    if _p not in sys.path:
        sys.path.append(_p)

from contextlib import ExitStack

import numpy as np

import concourse.bass as bass  # noqa: F401
import concourse.tile as tile
from concourse import bacc, mybir
from concourse.bass_utils import run_bass_kernel_spmd

F32 = mybir.dt.float32
F32R = mybir.dt.float32r
BF16 = mybir.dt.bfloat16
P = 128
KC = 128
QT = 512
N_CORES = 8
HD = 64
NEG = -1e9

TRACE = False            # test harness flips this for NTFF profiling
TRACE_CORES = None
LAST_RESULT = None       # BassKernelResults of the last run (for the harness)

_nc_cache = {}


def build_attn_core(S=2048, H=2048, NH=8, mask_mode="causal", debug_dump=False):
    """Build + bass-compile the per-core program (one batch element).

    DRAM inputs (per core):
      ht  [H, S] f32r    hidden transposed       wq [H, NH*HD] f32r (pre-scaled,
                                                 head-permuted: chunk cc = heads
                                                 (cc, 4+cc))
      wkv [H, 4*HD] f32r [K0|K1|V0|V1]           wo [NH*HD, H] f32r (row-permuted)
      trif [KC, QT] f32r [tri | zeros]: trif[k,j] = -1e9 if k > j else 0
      identr [P, P] f32r identity                zrow [1, HD+1] f32r zeros
      maskt [S, S] f32   (mask_mode=="full" only) additive mask transposed
    Output: out_p [S, H] f32.
    """
    NKV = 2
    CL = NH * HD                       # 512 q cols
    NCC = CL // P                      # 4 qt chunks
    NHC = H // P                       # 16 contraction chunks
    NT = S // QT                       # 4 token tiles / q blocks
    KPB = S // KC                      # 16 k chunks
    DPT = QT // KC                     # 4 k chunks per q block
    assert NH == 8 and S % QT == 0 and H % P == 0

    nc = bacc.Bacc("TRN2", target_bir_lowering=False, debug=False)

    ht = nc.dram_tensor("ht", [H, S], F32R, kind="ExternalInput").ap()
    wq = nc.dram_tensor("wq", [H, CL], F32R, kind="ExternalInput").ap()
    wkv = nc.dram_tensor("wkv", [H, 4 * HD], F32R, kind="ExternalInput").ap()
    wo = nc.dram_tensor("wo", [CL, H], F32R, kind="ExternalInput").ap()
    trif = nc.dram_tensor("trif", [KC, QT], F32R, kind="ExternalInput").ap()
    identr = nc.dram_tensor("identr", [P, P], F32R, kind="ExternalInput").ap()
    zrow = nc.dram_tensor("zrow", [1, HD + 1], F32R, kind="ExternalInput").ap()
    if mask_mode == "full":
        maskt = nc.dram_tensor("maskt", [S, S], F32, kind="ExternalInput").ap()
    out_p = nc.dram_tensor("out_p", [S, H], F32, kind="ExternalOutput").ap()
    if debug_dump:
        dbg_qt = nc.dram_tensor("dbg_qt", [NCC, P, S], F32, kind="ExternalOutput").ap()
        dbg_kt = nc.dram_tensor("dbg_kt", [P, S], F32, kind="ExternalOutput").ap()
        dbg_v = nc.dram_tensor("dbg_v", [P, S // KC, NKV, HD + 1], F32,
                               kind="ExternalOutput").ap()
        dbg_ctx = nc.dram_tensor("dbg_ctx", [2, P, NCC, QT], F32,
                                 kind="ExternalOutput").ap()

    with tile.TileContext(nc) as tc, ExitStack() as ctx:
        # ---- persistent SBUF ----
        pers = ctx.enter_context(tc.tile_pool(name="pers", bufs=1))
        wq_sb = pers.tile([P, NHC, CL], F32R, tag="wq")
        wkv_sb = pers.tile([P, NHC, 4 * HD], F32R, tag="wkv")
        # DMA order matters: the first proj matmul needs only wq/wkv chunk 0
        # (+ its h_t tile) — emit those first so the PE starts ~2us in, and
        # the big wo_sb transfer last (not needed until attention ends).
        wq_r = wq.rearrange("(o p) m -> p o m", p=P)
        wkv_r = wkv.rearrange("(o p) m -> p o m", p=P)
        trif_sb = pers.tile([KC, QT], F32R, tag="trif")
        idr_sb = pers.tile([P, P], F32R, tag="identr")
        zrow_sb = pers.tile([1, HD + 1], F32R, tag="zrow")
        wo_sb = pers.tile([P, NCC, H], F32R, tag="wo")

        def late_weight_dmas():
            nc.sync.dma_start(trif_sb[:], trif)
            nc.sync.dma_start(idr_sb[:], identr)
            nc.sync.dma_start(zrow_sb[:], zrow)
            for oc in range(NCC):
                nc.sync.dma_start(wo_sb[:, oc], wo.rearrange(
                    "(o p) m -> p o m", p=P)[:, oc])

        # fp32 identity for PE transposes (gpsimd memset can write f32)
        ident = pers.tile([P, P], F32, tag="ident")
        nc.gpsimd.memset(ident[:], 1.0)
        nc.gpsimd.affine_select(
            out=ident[:], in_=ident[:],
            compare_op=mybir.AluOpType.is_equal, fill=0.0,
            base=0, pattern=[[-1, P]], channel_multiplier=1,
        )

        qt_sb = pers.tile([P, NCC, S], F32R, tag="qt")
        kt_sb = pers.tile([P, S], F32R, tag="kt")
        v_sb = pers.tile([P, KPB, NKV, HD + 1], BF16, tag="v")
        # ones column for the softmax denominator: fill everything with 1.0,
        # proj drains then overwrite cols 0..63
        nc.gpsimd.memset(v_sb[:], 1.0)

        # ---- pools ----
        hpool = ctx.enter_context(tc.tile_pool(name="hpool", bufs=4))
        vtmp_pool = ctx.enter_context(tc.tile_pool(name="vtmp", bufs=2))
        pt_pool = ctx.enter_context(tc.tile_pool(name="pt", bufs=4))
        npool = ctx.enter_context(tc.tile_pool(name="npool", bufs=3))
        cxpool = ctx.enter_context(tc.tile_pool(name="cxpool", bufs=2))
        if mask_mode == "full":
            mpool = ctx.enter_context(tc.tile_pool(name="mpool", bufs=4))

        # PSUM: exactly 8 banks across both phases
        psS = ctx.enter_context(tc.tile_pool(name="psS", bufs=2, space="PSUM"))
        psC = ctx.enter_context(tc.tile_pool(name="psC", bufs=2, space="PSUM"))
        psO = ctx.enter_context(tc.tile_pool(name="psO", bufs=2, space="PSUM"))

        ctx_sb = [cxpool.tile([P, NCC, QT], F32R, tag="ctx", name=f"ctx{i}")
                  for i in range(2)]

        # ================= projections (one q tile) =================
        # psS slots ([128,1024] = 2 banks) hold Q chunk pairs; psO slots
        # ([128,512]) hold the K and V chunks; psC slots host the transposes.
        # Tiles >= 1 are emitted interleaved with attention of tile-1 (their
        # drains go on DVE; Act is the attention-window co-bottleneck).
        def proj_tile(t):
            q0 = t * QT
            pq01 = psS.tile([P, 2 * QT], F32, tag="sps", name="pq01")
            pq23 = psS.tile([P, 2 * QT], F32, tag="sps", name="pq23")
            pk = psO.tile([P, QT], F32, tag="po", name="pk")
            pv = psO.tile([P, QT], F32, tag="po", name="pv")
            for hc in range(NHC):
                if t == 0:
                    # stream the weight chunks in with the first tile so the
                    # PE starts ~2us in instead of waiting for all weights
                    nc.sync.dma_start(wq_sb[:, hc], wq_r[:, hc])
                    nc.sync.dma_start(wkv_sb[:, hc], wkv_r[:, hc])
                h_t = hpool.tile([P, QT], F32R, tag="h")
                nc.sync.dma_start(h_t[:], ht[hc * P:(hc + 1) * P, q0:q0 + QT])
                fl = dict(start=(hc == 0), stop=(hc == NHC - 1))
                nc.tensor.matmul(pq01[:, :QT], wq_sb[:, hc, 0:P], h_t[:], **fl)
                nc.tensor.matmul(pq01[:, QT:], wq_sb[:, hc, P:2 * P], h_t[:], **fl)
                nc.tensor.matmul(pq23[:, :QT], wq_sb[:, hc, 2 * P:3 * P], h_t[:], **fl)
                nc.tensor.matmul(pq23[:, QT:], wq_sb[:, hc, 3 * P:4 * P], h_t[:], **fl)
                nc.tensor.matmul(pk[:], wkv_sb[:, hc, 0:P], h_t[:], **fl)
                nc.tensor.matmul(pv[:], wkv_sb[:, hc, P:2 * P], h_t[:], **fl)
            cp = nc.scalar.copy if t == 0 else nc.vector.tensor_copy
            cp(qt_sb[:, 0, q0:q0 + QT], pq01[:, :QT])
            cp(qt_sb[:, 1, q0:q0 + QT], pq01[:, QT:])
            cp(qt_sb[:, 2, q0:q0 + QT], pq23[:, :QT])
            cp(qt_sb[:, 3, q0:q0 + QT], pq23[:, QT:])
            cp(kt_sb[:, q0:q0 + QT], pk[:])
            vtmp = vtmp_pool.tile([P, QT], F32, tag="vt")
            nc.vector.tensor_copy(vtmp[:], pv[:])
            # V_T chunk -> PE-transpose into v_sb (natural [k, d] layout)
            for j in range(NKV):
                for s4 in range(DPT):
                    tp = psC.tile([P, HD], F32, tag="cps", name="tp")
                    nc.tensor.transpose(
                        tp[:, :HD],
                        vtmp[HD * j:HD * (j + 1), s4 * KC:(s4 + 1) * KC],
                        ident[HD * j:HD * (j + 1), HD * j:HD * (j + 1)],
                    )
                    nc.vector.tensor_copy(v_sb[:, t * DPT + s4, j, :HD], tp[:, :HD])

        def debug_dumps():
            for c in range(NCC):
                nc.sync.dma_start(dbg_qt[c], qt_sb[:, c, :].bitcast(F32))
            nc.sync.dma_start(dbg_kt[:], kt_sb[:].bitcast(F32))
            dpool = ctx.enter_context(tc.tile_pool(name="dpool", bufs=1))
            dvt = dpool.tile([P, KPB, NKV, HD + 1], F32, tag="dvt")
            nc.vector.tensor_copy(dvt[:], v_sb[:])
            nc.sync.dma_start(dbg_v[:], dvt[:])

        # ================= Phase B: attention, Wo lagged one q-block =======
        def emit_wo(qtb):
            src = ctx_sb[qtb % 2]
            q0 = qtb * QT
            for tc_ in range(QT // P):
                for et in range(H // QT):
                    po = psO.tile([P, QT], F32, tag="po", name="po")
                    for cc in range(NCC):
                        nc.tensor.matmul(
                            po[:],
                            src[:, cc, tc_ * P:(tc_ + 1) * P],
                            wo_sb[:, cc, et * QT:(et + 1) * QT],
                            start=(cc == 0), stop=(cc == NCC - 1),
                        )
                    ob = npool.tile([P, QT], F32, tag="ob")
                    nc.vector.tensor_copy(ob[:], po[:])
                    nc.sync.dma_start(
                        out_p[q0 + tc_ * P:q0 + (tc_ + 1) * P,
                              et * QT:(et + 1) * QT],
                        ob[:],
                    )

        def chunk_pairs(qtb):
            if mask_mode == "causal":
                # (kc, local q offset, extent, in-tile offset); diagonal
                # chunks first, paired (d0,d3) and (d1,d2) so each pair's
                # exp covers ONE contiguous range
                d = qtb * DPT
                pairs = [
                    [(d + 0, 0, QT, 0), (d + 3, 3 * KC, KC, QT)],
                    [(d + 1, KC, QT - KC, KC), (d + 2, 2 * KC, 2 * KC, QT)],
                ]
                ints = [(kc, 0, QT, (i % 2) * QT)
                        for i, kc in enumerate(range(qtb * DPT))]
            else:
                ints = [(kc, 0, QT, (i % 2) * QT)
                        for i, kc in enumerate(range(KPB))]
                pairs = []
            return pairs + [ints[i:i + 2] for i in range(0, len(ints), 2)]

        def emit_pair(qtb, h, cps, pair, first, lastp):
            q0 = qtb * QT
            r, cc = h // 4, h % 4
            rows = slice(HD * r, HD * (r + 1))
            sps = psS.tile([P, 2 * QT], F32, tag="sps", name="sps")
            pt = pt_pool.tile([P, 2 * QT], BF16, tag="pt")
            for kc, qoff, ext, off in pair:
                diag = mask_mode == "causal" and kc >= qtb * DPT
                if diag:
                    # tri mask first (start=True zeroes the region),
                    # then accumulate the scores on top
                    nc.tensor.matmul(
                        sps[:, off:off + ext], idr_sb[:],
                        trif_sb[:, :ext], start=True, stop=False)
                nc.tensor.matmul(
                    sps[:, off:off + ext],
                    kt_sb[rows, kc * KC:(kc + 1) * KC],
                    qt_sb[rows, cc, q0 + qoff:q0 + qoff + ext],
                    start=not diag, stop=True,
                )
                if mask_mode == "full":
                    mt = mpool.tile([KC, QT], F32, tag="mt")
                    nc.sync.dma_start(
                        mt[:], maskt[kc * KC:(kc + 1) * KC, q0:q0 + QT])
                    nc.vector.tensor_add(sps[:, off:off + ext],
                                         sps[:, off:off + ext], mt[:])
            # one exp over the pair's contiguous range
            lo = min(off for _, _, _, off in pair)
            hi = max(off + ext for _, _, ext, off in pair)
            nc.scalar.activation(pt[:, lo:hi], sps[:, lo:hi],
                                 mybir.ActivationFunctionType.Exp)
            for ci, (kc, qoff, ext, off) in enumerate(pair):
                nc.tensor.matmul(
                    cps[:, qoff:qoff + ext],
                    v_sb[:, kc, r, :],
                    pt[:, off:off + ext],
                    start=first and ci == 0,
                    stop=lastp and ci == len(pair) - 1 and qoff == 0,
                )

        def finish_head(qtb, h, cps):
            q0 = qtb * QT
            r, cc = h // 4, h % 4
            if mask_mode == "causal" and qtb == 0:
                # no full-width final chunk at qtb 0; close the accumulation
                # group over the whole bank with a zero matmul
                nc.tensor.matmul(
                    cps[:], zrow_sb[:], qt_sb[0:1, 0, q0:q0 + QT],
                    start=False, stop=True)
            # ---- normalize: ctx[:64] * recip(denom row 64) ----
            recip = npool.tile([P, QT + 2 * (QT // P)], F32, tag="recip")
            DW = QT // P
            # reshape the [1, QT] denom row to [P, QT/P] via DMA so the
            # partition-serial reciprocal runs 128-wide (~100ns vs ~4us)
            nc.vector.tensor_copy(recip[HD:HD + 1, :QT], cps[HD:HD + 1, :])
            nc.sync.dma_start(recip[:, QT:QT + DW], recip[HD:HD + 1, :QT])
            nc.vector.reciprocal(recip[:, QT + DW:], recip[:, QT:QT + DW])
            nc.sync.dma_start(recip[0:1, :QT], recip[:, QT + DW:])
            bcast = npool.tile([HD, QT], F32, tag="bcast")
            nc.gpsimd.partition_broadcast(bcast[:], recip[0:1, :QT])
            dst = ctx_sb[qtb % 2]
            if r == 0:
                nc.vector.tensor_mul(dst[:HD, cc, :], cps[:HD, :], bcast[:])
            else:
                ctmp = npool.tile([HD, QT], F32R, tag="ctmp")
                nc.vector.tensor_mul(ctmp[:], cps[:HD, :], bcast[:])
                nc.sync.dma_start(dst[HD:2 * HD, cc, :], ctmp[:])

        def attn_head_pair(qtb, ha, hb):
            # interleave two heads' chunk streams so independent work sits
            # between each exp and the matmuls that consume it (keeps the
            # in-order PE queue from head-of-line blocking on Act latency)
            pairs = chunk_pairs(qtb)
            cpsA = psC.tile([HD + 1, QT], F32, tag="cps", name="cpsA")
            cpsB = psC.tile([HD + 1, QT], F32, tag="cps", name="cpsB")
            for p, pair in enumerate(pairs):
                emit_pair(qtb, ha, cpsA, pair, p == 0, p == len(pairs) - 1)
                emit_pair(qtb, hb, cpsB, pair, p == 0, p == len(pairs) - 1)
            finish_head(qtb, ha, cpsA)
            finish_head(qtb, hb, cpsB)

        proj_tile(0)
        late_weight_dmas()
        for qtb in range(NT):
            for hp in range(0, NH, 2):
                attn_head_pair(qtb, hp, hp + 1)
            if debug_dump and qtb < 2:
                nc.sync.dma_start(dbg_ctx[qtb], ctx_sb[qtb % 2][:].bitcast(F32))
            if qtb > 0:
                emit_wo(qtb - 1)
            if qtb + 1 < NT:
                proj_tile(qtb + 1)
        emit_wo(NT - 1)
        if debug_dump:
            debug_dumps()

    nc.compile()
    return nc


def _detect_mask_mode(m, S):
    if not np.any(m):
        return "zeros"
    b0 = np.asarray(m[0, 0])
    qi = np.arange(S)
    tl = qi[None, :] <= qi[:, None]
    if (b0[tl] == 0.0).all() and (b0[~tl] <= -1e8).all() and (m == b0).all():
        return "causal"
    return "full"


def shard_inputs(hidden_states, attention_mask, Wq, Wk, Wv, Wo, mask_mode):
    B, S, H = hidden_states.shape
    NH = Wq.shape[1] // HD             # 32 total
    NKV = Wk.shape[1] // HD            # 8 total
    G = 4                              # head-groups (cores per batch)
    NHL = NH // G                      # 8 q heads per core
    scale = np.float32(1.0 / np.sqrt(HD))

    # permutation: chunk cc holds q-heads (cc, 4+cc) -> [0,4,1,5,2,6,3,7]
    perm = [h for cc in range(NHL // 2) for h in (cc, cc + NHL // 2)]

    trif = np.zeros((KC, QT), np.float32)
    ki, qj = np.meshgrid(np.arange(KC), np.arange(KC), indexing="ij")
    trif[:, :KC] = np.where(ki > qj, NEG, 0.0).astype(np.float32)
    identr = np.eye(P, dtype=np.float32)
    zr = np.zeros((1, HD + 1), np.float32)

    in_maps = []
    for c in range(N_CORES):
        b, g = divmod(c, G)
        ht = np.ascontiguousarray(
            np.asarray(hidden_states[b]).T.astype(np.float32))
        heads = [g * NHL + perm[i] for i in range(NHL)]
        wq_c = np.ascontiguousarray(np.concatenate(
            [Wq[:, h * HD:(h + 1) * HD] for h in heads], axis=1)
            .astype(np.float32) * scale)
        kv0 = g * 2
        wkv_c = np.ascontiguousarray(np.concatenate(
            [Wk[:, kv0 * HD:(kv0 + 2) * HD],
             Wv[:, kv0 * HD:(kv0 + 2) * HD]], axis=1).astype(np.float32))
        wo_c = np.ascontiguousarray(np.concatenate(
            [Wo[h * HD:(h + 1) * HD, :] for h in heads], axis=0)
            .astype(np.float32))
        im = {"ht": ht, "wq": wq_c, "wkv": wkv_c, "wo": wo_c,
              "trif": trif, "identr": identr, "zrow": zr}
        if mask_mode == "full":
            im["maskt"] = np.ascontiguousarray(
                np.asarray(attention_mask)[b, 0].T.astype(np.float32))
        in_maps.append(im)
    return in_maps


def kernel(hidden_states, attention_mask, Wq, Wk, Wv, Wo):
    global LAST_RESULT
    hidden_states = np.asarray(hidden_states, dtype=np.float32)
    attention_mask = np.asarray(attention_mask, dtype=np.float32)
    Wq, Wk, Wv, Wo = (np.asarray(w, dtype=np.float32) for w in (Wq, Wk, Wv, Wo))
    B, S, H = hidden_states.shape

    mask_mode = _detect_mask_mode(attention_mask, S)
    in_maps = shard_inputs(hidden_states, attention_mask, Wq, Wk, Wv, Wo,
                           mask_mode)

    key = (B, S, H, mask_mode)
    if key not in _nc_cache:
        _nc_cache[key] = build_attn_core(S=S, H=H, NH=8, mask_mode=mask_mode)
    nc = _nc_cache[key]

    res = run_bass_kernel_spmd(nc, in_maps, core_ids=list(range(N_CORES)),
                               trace=TRACE, trace_cores=TRACE_CORES)
    LAST_RESULT = res

    out = np.zeros((B, S, H), np.float32)
    for c in range(N_CORES):
        out[c // 4] += res.results[c]["out_p"]
    return out
